# revision 4
# baseline (speedup 1.0000x reference)
"""ConceptNet retrieval-KNN kernel for 8 Trainium2 NeuronCores.

Strategy (sharding_hint): shard train_embeddings over N across the 8 cores.
Each core streams its (1024, 25600) shard once from HBM (memory roofline),
computing score[c, n] = 2*<concept_c, te_n> - |te_n|^2 via two accumulating
fp32r matmul passes (stationary = 2*concept chunk, then stationary = -1s with
moving te^2).  The score tile is transposed (TensorE) into a per-concept
(128, 200) layout so a single vector-engine max8/max_index pair per concept
extracts the top-8 candidates per 200-column cell.  The (val, idx) candidate
lists (8 cores x 128 cells x 8) are reduced to the global top-k on the host
(tiny), with an exact host-side fallback if any cell saturates.  The two (B,4)
predictions are computed on-device from a fused (8, D) weight matrix
[hx_weight; hx_weight @ proj], data-parallel over the batch.  The remaining
scalar outputs only involve the (C, C) gram matrix and the k*C selected dot
products - negligible host work.
"""

import os
import sys

sys.path.insert(0, "/opt/trn_rl_repo")

import numpy as np

D = 1024
N = 200000
C = 50
B = 4096
NCLS = 4
NCORES = 8
NSH_REAL = N // NCORES          # 25000 real columns per shard
NTILE = 512
NTILES = 50
NSH = NTILE * NTILES            # 25600 padded shard width
PAD_VAL = 1000.0                # pad columns get score ~ -1e9, never selected
NSEG = NTILES * 4               # 200 columns per (core, partition) cell
BSH = B // NCORES               # 512 batch rows per core
DCH = D // 128                  # 8 contraction chunks

_program = None
last_exec_time_ns = None
last_results = None


def _build_program():
    import concourse.bacc as bacc
    import concourse.tile as tile
    from concourse import mybir

    f32 = mybir.dt.float32
    f32r = mybir.dt.float32r
    u32 = mybir.dt.uint32
    AF = mybir.ActivationFunctionType

    nc = bacc.Bacc("TRN2", target_bir_lowering=False, debug=False,
                   num_devices=NCORES)
    te = nc.dram_tensor("te", [D, NSH], f32r, kind="ExternalInput").ap()
    conc2 = nc.dram_tensor("conc2", [128, DCH, C], f32r, kind="ExternalInput").ap()
    negs = nc.dram_tensor("negs", [128, C], f32r, kind="ExternalInput").ap()
    ident = nc.dram_tensor("ident", [C, C], f32, kind="ExternalInput").ap()
    tebt = nc.dram_tensor("tebt", [128, DCH, BSH], f32, kind="ExternalInput").ap()
    w2t = nc.dram_tensor("w2t", [128, DCH, 8], f32, kind="ExternalInput").ap()

    cand_val = nc.dram_tensor("cand_val", [128, C * 8], f32, kind="ExternalOutput").ap()
    cand_idx = nc.dram_tensor("cand_idx", [128, C * 8], u32, kind="ExternalOutput").ap()
    bpred = nc.dram_tensor("bpred", [8, BSH], f32, kind="ExternalOutput").ap()

    ter = te.rearrange("(d p) n -> p d n", p=128)

    with tile.TileContext(nc) as tc:
        with tc.tile_pool(name="const", bufs=1) as constp, \
             tc.tile_pool(name="tep", bufs=3) as tep, \
             tc.tile_pool(name="sqp", bufs=3) as sqp, \
             tc.tile_pool(name="scp", bufs=3) as scp, \
             tc.tile_pool(name="big", bufs=1) as bigp, \
             tc.tile_pool(name="pscore", bufs=2, space="PSUM") as psp, \
             tc.tile_pool(name="ptrans", bufs=2, space="PSUM") as pstp, \
             tc.tile_pool(name="pb", bufs=1, space="PSUM") as psbp:

            conc2_sb = constp.tile([128, DCH, C], f32r)
            nc.sync.dma_start(conc2_sb[:], conc2)
            ident_sb = constp.tile([C, C], f32)
            nc.sync.dma_start(ident_sb[:], ident)
            w2t_sb = constp.tile([128, DCH, 8], f32)
            nc.sync.dma_start(w2t_sb[:], w2t)
            tebt_sb = constp.tile([128, DCH, BSH], f32)
            nc.sync.dma_start(tebt_sb[:], tebt)
            negones = constp.tile([128, C], f32r)
            nc.sync.dma_start(negones[:], negs)

            # batch predictions: true fp32 matmul (precision), tiny cost
            psb = psbp.tile([8, BSH], f32)
            for d in range(DCH):
                nc.tensor.matmul(psb[:], w2t_sb[:, d, :], tebt_sb[:, d, :],
                                 start=(d == 0), stop=(d == DCH - 1))
            bsb = constp.tile([8, BSH], f32)
            nc.scalar.activation(bsb[:], psb[:], AF.Copy)
            nc.sync.dma_start(bpred, bsb[:])

            score_t = bigp.tile([128, C * NSEG], f32)
            score_v = score_t[:].rearrange("p (c m) -> p c m", c=C)
            for j in range(NTILES):
                te_t = tep.tile([128, DCH, NTILE], f32r)
                nc.sync.dma_start(te_t[:], ter[:, :, j * NTILE:(j + 1) * NTILE])
                sq_t = sqp.tile([128, DCH, NTILE], f32r)
                nc.scalar.activation(sq_t[:, 0:4, :], te_t[:, 0:4, :], AF.Square)
                nc.vector.tensor_mul(sq_t[:, 4:8, :], te_t[:, 4:8, :],
                                     te_t[:, 4:8, :])
                ps = psp.tile([C, NTILE], f32)
                for d in range(DCH):
                    nc.tensor.matmul(ps[:], conc2_sb[:, d, :],
                                     te_t[:, d, :],
                                     start=(d == 0), stop=False)
                for d in range(DCH):
                    nc.tensor.matmul(ps[:], negones[:],
                                     sq_t[:, d, :],
                                     start=False, stop=(d == DCH - 1))
                sc = scp.tile([C, NTILE], f32)
                nc.scalar.activation(sc[:], ps[:], AF.Copy)
                pst = pstp.tile([128, 4, C], f32)
                for s in range(4):
                    nc.tensor.transpose(pst[:, s, :],
                                        sc[:, s * 128:(s + 1) * 128],
                                        ident_sb[:])
                nc.scalar.activation(score_v[:, :, j * 4:(j + 1) * 4],
                                     pst[:].transpose([0, 2, 1]), AF.Copy)

            val_t = bigp.tile([128, C * 8], f32)
            idx_t = bigp.tile([128, C * 8], u32)
            for c in range(C):
                nc.vector.max(val_t[:, c * 8:(c + 1) * 8],
                              score_t[:, c * NSEG:(c + 1) * NSEG])
                nc.vector.max_index(idx_t[:, c * 8:(c + 1) * 8],
                                    val_t[:, c * 8:(c + 1) * 8],
                                    score_t[:, c * NSEG:(c + 1) * NSEG])
            nc.sync.dma_start(cand_val, val_t[:])
            nc.sync.dma_start(cand_idx, idx_t[:])

    nc.compile()
    return nc


def kernel(**inputs):
    global _program, last_exec_time_ns, last_results

    concept = np.asarray(inputs["concept"], dtype=np.float32)        # (D, C)
    TE = np.asarray(inputs["train_embeddings"], dtype=np.float32)    # (D, N)
    te_b = np.asarray(inputs["train_embedding"], dtype=np.float32)   # (B, D)
    hxw = np.asarray(inputs["hx_weight"], dtype=np.float32)          # (4, D)
    hxb = np.asarray(inputs["hx_bias"], dtype=np.float32)            # (4,)
    k = int(np.asarray(inputs["topk"]))

    from concourse.bass_utils import run_bass_kernel_spmd

    if _program is None:
        _program = _build_program()
    nc = _program

    # ---- tiny host math (f64): gram, projection weights ----
    c64 = concept.astype(np.float64)
    gram = c64.T @ c64                                              # (C, C)
    W_proj = ((hxw.astype(np.float64) @ c64) @ np.linalg.inv(gram)) @ c64.T
    W2 = np.concatenate([hxw.astype(np.float64), W_proj], axis=0)   # (8, D)
    W2 = np.ascontiguousarray(W2.astype(np.float32))

    # ---- per-core input maps ----
    conc2_host = np.ascontiguousarray(
        (2.0 * concept).reshape(DCH, 128, C).transpose(1, 0, 2))
    ident_host = np.eye(C, dtype=np.float32)
    negs_host = np.full((128, C), -1.0, dtype=np.float32)
    w2t_host = np.ascontiguousarray(
        W2.T.reshape(DCH, 128, 8).transpose(1, 0, 2))

    in_maps = []
    for cid in range(NCORES):
        shard = np.full((D, NSH), PAD_VAL, dtype=np.float32)
        shard[:, :NSH_REAL] = TE[:, cid * NSH_REAL:(cid + 1) * NSH_REAL]
        tb = np.ascontiguousarray(te_b[cid * BSH:(cid + 1) * BSH].T)  # (D, BSH)
        tebt_host = np.ascontiguousarray(
            tb.reshape(DCH, 128, BSH).transpose(1, 0, 2))
        in_maps.append({
            "te": shard,
            "negs": negs_host,
            "conc2": conc2_host,
            "ident": ident_host,
            "tebt": tebt_host,
            "w2t": w2t_host,
        })

    trace = os.environ.get("CONCEPTNET_TRACE", "0") == "1"
    res = run_bass_kernel_spmd(nc, in_maps, list(range(NCORES)), trace=trace)
    last_exec_time_ns = res.exec_time_ns
    last_results = res

    # ---- batch predictions ----
    orig_pred = np.empty((B, NCLS), dtype=np.float32)
    y_pred = np.empty((B, NCLS), dtype=np.float32)
    for cid in range(NCORES):
        bp = res.results[cid]["bpred"]                              # (8, BSH)
        orig_pred[cid * BSH:(cid + 1) * BSH] = bp[0:NCLS].T + hxb
        y_pred[cid * BSH:(cid + 1) * BSH] = bp[NCLS:2 * NCLS].T + hxb

    # ---- global top-k reduce on host ----
    vals = np.stack([res.results[cid]["cand_val"] for cid in range(NCORES)])
    idxs = np.stack([res.results[cid]["cand_idx"] for cid in range(NCORES)])
    vals = vals.reshape(NCORES, 128, C, 8)
    m = idxs.reshape(NCORES, 128, C, 8).astype(np.int64)
    p = np.arange(128, dtype=np.int64)[None, :, None, None]
    local_col = (m // 4) * NTILE + (m % 4) * 128 + p
    core = np.arange(NCORES, dtype=np.int64)[:, None, None, None]
    gcol = core * NSH_REAL + local_col
    valid = local_col < NSH_REAL

    # (C, ncells, 8) with ncells = NCORES * 128
    v_f = np.where(valid, vals, -np.inf).transpose(2, 0, 1, 3).reshape(C, -1, 8)
    g_f = gcol.transpose(2, 0, 1, 3).reshape(C, -1, 8)
    ncand = v_f.shape[1] * 8

    # duplicate-index detection per cell (fp-tie artifact of max_index);
    # invalid (pad) slots get unique negative ids so they never look duplicated
    uid = -np.arange(v_f.size, dtype=np.int64).reshape(v_f.shape) - 1
    g_sorted = np.sort(np.where(np.isneginf(v_f), uid, g_f), axis=2)
    cell_dup = (np.diff(g_sorted, axis=2) == 0).any(axis=2)         # (C, ncells)

    sel_cols = np.empty((C, k), dtype=np.int64)
    need_fallback = []
    vf_flat = v_f.reshape(C, ncand)
    gf_flat = g_f.reshape(C, ncand)
    for c in range(C):
        order = np.argpartition(-vf_flat[c], k - 1)[:k]
        cells = order // 8
        cnt = np.bincount(cells, minlength=v_f.shape[1])
        sel = gf_flat[c][order]
        if (cnt >= 8).any() or cell_dup[c][cells].any() or \
                len(np.unique(sel)) < k:
            need_fallback.append(c)
            continue
        sel_cols[c] = sel

    if need_fallback:
        te_sq_full = np.einsum("dn,dn->n", TE, TE)
        for c in need_fallback:
            scores = 2.0 * (concept[:, c] @ TE) - te_sq_full
            sel_cols[c] = np.argpartition(-scores, k - 1)[:k]

    # ---- L_sparse_1 from selected neighbor dot products ----
    selected = TE[:, sel_cols.reshape(-1)].reshape(D, C, k)
    L1 = np.einsum("dck,dc->", selected.astype(np.float64), c64) / (k * C)

    # ---- gram-based scalars ----
    eye = np.eye(C, dtype=np.float64)
    L2 = (gram * (1.0 - eye)).mean()
    nm = (gram * eye).mean()
    sp = np.abs(gram - eye).mean()

    return (orig_pred, y_pred,
            np.float32(L1), np.float32(L2), np.float32(nm), np.float32(sp))
